# revision 1
# baseline (speedup 1.0000x reference)
"""GCN layer (message passing) on 8 Trainium2 NeuronCores via Bass/Tile.

    m = (h @ W) * norm            # [N, D] per-source messages
    n = segment_sum(m[src], dst)  # scatter-sum over E edges
    out = leaky_relu(n * norm + bias, 0.2)

Strategy (fully SPMD, no collectives):
  - Nodes sharded: core c owns dst rows [c*12500, (c+1)*12500).
  - Edges partitioned by dst owner on the host; within a core, grouped into
    49 groups of 256 dst nodes; within a group, bucketed by src//25000 so
    gather indices fit int16 for the custom `dma_gather` SWDGE instruction.
  - Device per group: dma_gather h[src] rows -> SBUF; build a scaled one-hot
    matrix OH[p,j] = (local_dst[p]==j)*norm[src[p]] in one DVE op; accumulate
    S[f,j] = sum_e h[src_e,f]*norm[src_e]*1[dst_e==j] via f32r matmuls into
    PSUM; apply W with a second f32r matmul; epilogue = *norm[dst] + bias +
    leaky_relu.  Output written feature-major [D, nodes]; host transposes.
  - Algebra: sum(m[src]) = W.T @ sum(h[src]*norm[src]) — W applied once per
    256-node group instead of per edge, so the matmul cost is E-independent.
"""

import sys

if "/opt/trn_rl_repo" not in sys.path:
    sys.path.insert(0, "/opt/trn_rl_repo")

import numpy as np
import ml_dtypes

import concourse.bass as bass
import concourse.bacc as bacc
import concourse.mybir as mybir
import concourse.tile as tile
from concourse.bass_utils import run_bass_kernel_spmd

P = 128
N = 100000
E = 1600000
D = 128
NCORES = 8
NODES_PER_CORE = N // NCORES  # 12500
GN = 256  # dst nodes per group
G = (NODES_PER_CORE + GN - 1) // GN  # 49 groups per core
NBUCK = 4  # src buckets (int16 index range)
BUCK = 25000  # bucket size; max local idx 24999 < 32767


def configure(n, e, nbuck=None):
    """Adjust problem-size globals (for scale bisection in testing)."""
    global N, E, NODES_PER_CORE, G, NBUCK, BUCK
    N = n
    E = e
    NODES_PER_CORE = N // NCORES
    G = (NODES_PER_CORE + GN - 1) // GN
    NBUCK = nbuck if nbuck is not None else 4
    BUCK = (N + NBUCK - 1) // NBUCK
    assert BUCK <= 32767

f32 = mybir.dt.float32
f32r = mybir.dt.float32r
i16 = mybir.dt.int16


def _f32r_round(x: np.ndarray) -> np.ndarray:
    """Round to the f32r (bf16 hi+lo pair) grid, as the PE assumes."""
    hi = x.astype(ml_dtypes.bfloat16).astype(np.float32)
    lo = (x - hi).astype(ml_dtypes.bfloat16).astype(np.float32)
    return hi + lo


def _prep_core(src_c, dst_c, norm, core):
    """Slot this core's edges: edge k of its (group,bucket) run lands at
    partition k%128, slot off_gb + k//128. Returns per-(g,b) counts plus
    the raw per-edge (group, bucket, rank) so arrays can be filled once
    the cross-core padded sizes are known."""
    ldst = dst_c - core * NODES_PER_CORE
    grp = ldst // GN
    buck = src_c // BUCK
    key = grp * NBUCK + buck
    # sort by (group, bucket), then by src within each run: ascending gather
    # addresses give the SDMA engines HBM row-buffer locality
    order = np.lexsort((src_c, key))
    key_s = key[order]
    counts = np.bincount(key_s, minlength=G * NBUCK)
    starts = np.zeros(G * NBUCK + 1, np.int64)
    np.cumsum(counts, out=starts[1:])
    rank = np.arange(len(key_s)) - starts[key_s]
    return order, key_s, rank, counts


def build_host_data(h, norm, weight, bias, src, dst):
    """All sharding/layout prep. Returns (in_maps, meta) for the SPMD run."""
    norm1 = np.ascontiguousarray(norm).reshape(-1)
    owner = dst // NODES_PER_CORE

    cores = []
    counts_all = np.zeros((NCORES, G * NBUCK), np.int64)
    for c in range(NCORES):
        sel = owner == c
        src_c = src[sel]
        dst_c = dst[sel]
        order, key_s, rank, counts = _prep_core(src_c, dst_c, norm1, c)
        cores.append((src_c[order], dst_c[order], key_s, rank))
        counts_all[c] = counts

    # shared (across cores) padded slot counts per (g, b)
    s_gb = (counts_all.max(axis=0).reshape(G, NBUCK) + 127) // 128  # slots
    s_gb = np.maximum(s_gb, 1)  # at least one slot so every gather is valid
    nidx_gb = s_gb * 128
    slot_off = np.zeros((G, NBUCK), np.int64)  # slot offset of bucket within group
    for g in range(G):
        slot_off[g] = np.cumsum(np.concatenate([[0], s_gb[g][:-1]]))
    s_g = s_gb.sum(axis=1)  # total slots per group
    SMAX = int(s_g.max())
    icols_gb = nidx_gb // 16
    icol_off = np.zeros((G, NBUCK), np.int64)
    for g in range(G):
        icol_off[g] = np.cumsum(np.concatenate([[0], icols_gb[g][:-1]]))
    ic_g = icols_gb.sum(axis=1)
    ICMAX = int(ic_g.max())

    h_r = _f32r_round(np.ascontiguousarray(h))
    w_r = _f32r_round(np.ascontiguousarray(weight))
    bias_col = np.ascontiguousarray(bias).reshape(D, 1).astype(np.float32)
    iota = np.tile(np.arange(GN, dtype=np.float32)[None, :], (P, 1))

    in_maps = []
    for c in range(NCORES):
        src_s, dst_s, key_s, rank = cores[c]
        g_s = key_s // NBUCK
        b_s = key_s % NBUCK
        part = rank % 128
        slot = slot_off[g_s, b_s] + rank // 128

        lofs = np.full((G, P, SMAX), -1.0, np.float32)
        nsrc = np.zeros((G, P, SMAX), np.float32)
        lofs[g_s, part, slot] = (dst_s - c * NODES_PER_CORE - g_s * GN).astype(
            np.float32
        )
        nsrc[g_s, part, slot] = norm1[src_s]

        # int16 wrapped gather indices: idx k of a (g,b) run -> [k%16, k//16]
        idxw = np.zeros((G, 16, ICMAX * 16 // 16), np.int16)  # [G, 16, ICMAX]
        loc = (src_s - b_s * BUCK).astype(np.int16)
        col = icol_off[g_s, b_s] * 16 + rank  # linear idx position within group
        idxw[g_s, col % 16, col // 16] = loc
        idxw_full = np.broadcast_to(idxw[:, None, :, :], (G, 8, 16, ICMAX)).reshape(
            G, 128, ICMAX
        )

        ngrp = np.zeros((G, GN), np.float32)
        nv = norm1[c * NODES_PER_CORE : (c + 1) * NODES_PER_CORE]
        ngrp.reshape(-1)[: NODES_PER_CORE] = nv

        in_maps.append(
            {
                "h": h_r,
                "w": w_r,
                "bias_col": bias_col,
                "iota": iota,
                "gidxw": np.ascontiguousarray(idxw_full),
                "lofs": lofs,
                "nsrc": nsrc,
                "ngrp": ngrp,
            }
        )

    meta = {
        "s_gb": s_gb,
        "slot_off": slot_off,
        "s_g": s_g,
        "SMAX": SMAX,
        "icols_gb": icols_gb,
        "icol_off": icol_off,
        "ICMAX": ICMAX,
    }
    return in_maps, meta


def build_program(
    meta, repeats: int = 1, hg_bufs: int = 3, bufs1: bool = False, variant: str = "full"
):
    """Build the SPMD Bass program (same for all cores)."""
    s_gb = meta["s_gb"]
    slot_off = meta["slot_off"]
    s_g = meta["s_g"]
    SMAX = meta["SMAX"]
    icols_gb = meta["icols_gb"]
    icol_off = meta["icol_off"]
    ICMAX = meta["ICMAX"]

    if bufs1:
        hg_bufs = 1
    _b = (lambda x: 1) if bufs1 else (lambda x: x)
    nc = bacc.Bacc(
        "TRN2", target_bir_lowering=False, debug=False, num_devices=NCORES
    )
    h_d = nc.dram_tensor("h", [N, D], f32, kind="ExternalInput").ap()
    w_d = nc.dram_tensor("w", [D, D], f32, kind="ExternalInput").ap()
    bias_d = nc.dram_tensor("bias_col", [D, 1], f32, kind="ExternalInput").ap()
    iota_d = nc.dram_tensor("iota", [P, GN], f32, kind="ExternalInput").ap()
    gidx_d = nc.dram_tensor("gidxw", [G, P, ICMAX], i16, kind="ExternalInput").ap()
    lofs_d = nc.dram_tensor("lofs", [G, P, SMAX], f32, kind="ExternalInput").ap()
    nsrc_d = nc.dram_tensor("nsrc", [G, P, SMAX], f32, kind="ExternalInput").ap()
    ngrp_d = nc.dram_tensor("ngrp", [G, GN], f32, kind="ExternalInput").ap()
    outT_d = nc.dram_tensor("outT", [D, G * GN], f32, kind="ExternalOutput").ap()

    with tile.TileContext(nc) as tc:
        with (
            tc.tile_pool(name="consts", bufs=1) as consts,
            tc.tile_pool(name="meta_p", bufs=_b(3)) as meta_p,
            tc.tile_pool(name="gath", bufs=hg_bufs) as gath,
            tc.tile_pool(name="oh_p", bufs=_b(4)) as oh_p,
            tc.tile_pool(name="ep", bufs=_b(3)) as ep,
            tc.tile_pool(name="psum", bufs=_b(2), space="PSUM") as psum,
        ):
            w_sb = consts.tile([P, D], f32r)
            nc.sync.dma_start(out=w_sb[:], in_=w_d[:, :].bitcast(f32r))
            bias_sb = consts.tile([P, 1], f32)
            nc.sync.dma_start(out=bias_sb[:], in_=bias_d[:, :])
            iota_sb = consts.tile([P, GN], f32)
            nc.sync.dma_start(out=iota_sb[:], in_=iota_d[:, :])

            for _rep in range(repeats):
                for g in range(G):
                    SG = int(s_g[g])
                    ICG = int(icols_gb[g].sum())
                    idx_t = meta_p.tile([P, ICMAX], i16, tag="idx")
                    nc.sync.dma_start(
                        out=idx_t[:, :ICG], in_=gidx_d[g, :, :ICG]
                    )
                    lofs_t = meta_p.tile([P, SMAX], f32, tag="lofs")
                    nc.sync.dma_start(out=lofs_t[:, :SG], in_=lofs_d[g, :, :SG])
                    nsrc_t = meta_p.tile([P, SMAX], f32, tag="nsrc")
                    nc.sync.dma_start(out=nsrc_t[:, :SG], in_=nsrc_d[g, :, :SG])
                    ngrp_t = meta_p.tile([P, GN], f32, tag="ngrp")
                    ngrp_row = ngrp_d[g]
                    ngrp_bc = bass.AP(
                        tensor=ngrp_row.tensor,
                        offset=ngrp_row.offset,
                        ap=[[0, P]] + list(ngrp_row.ap),
                    )
                    nc.sync.dma_start(out=ngrp_t[:], in_=ngrp_bc)

                    hg = gath.tile([P, SMAX, D], f32r, tag="hg")
                    if variant == "compute":
                        # sequential bulk read of the same byte volume
                        nc.sync.dma_start(
                            out=hg[:, :SG, :],
                            in_=h_d[: SG * 128, :]
                            .bitcast(f32r)
                            .rearrange("(s p) d -> p s d", p=P),
                        )
                    else:
                        for b in range(NBUCK):
                            nb = int(s_gb[g, b]) * 128
                            so = int(slot_off[g, b])
                            co = int(icol_off[g, b])
                            nc.gpsimd.dma_gather(
                                hg[:, so : so + nb // 128, :],
                                h_d[BUCK * b :, :].bitcast(f32r),
                                idx_t[:, co : co + nb // 16],
                                nb,
                                nb,
                                D,
                                single_packet=(nb <= 1024),
                            )
                    if variant == "gather":
                        # consume every bucket's output cheaply (defeat DCE)
                        t2 = ep.tile([P, GN], f32, tag="t2")
                        for b in range(NBUCK):
                            so = int(slot_off[g, b])
                            nc.vector.tensor_copy(
                                out=t2[:, b * 4 : b * 4 + 4],
                                in_=hg[:, so, :4].bitcast(f32),
                            )
                        nc.sync.dma_start(
                            out=outT_d[:, g * GN : (g + 1) * GN], in_=t2[:]
                        )
                        continue

                    ps_S = psum.tile([P, GN], f32, space="PSUM", tag="ps_S")
                    for s in range(SG):
                        oh = oh_p.tile([P, GN], f32r, tag="oh")
                        nc.vector.scalar_tensor_tensor(
                            out=oh[:],
                            in0=iota_sb[:],
                            scalar=lofs_t[:, s : s + 1],
                            in1=nsrc_t[:, s : s + 1].to_broadcast((P, GN)),
                            op0=mybir.AluOpType.is_equal,
                            op1=mybir.AluOpType.mult,
                        )
                        nc.tensor.matmul(
                            out=ps_S[:],
                            lhsT=hg[:, s, :],
                            rhs=oh[:],
                            start=(s == 0),
                            stop=(s == SG - 1),
                        )

                    s_sb = ep.tile([P, GN], f32r, tag="s_sb")
                    nc.scalar.activation(
                        out=s_sb[:],
                        in_=ps_S[:],
                        func=mybir.ActivationFunctionType.Copy,
                    )

                    ps_O = psum.tile([P, GN], f32, space="PSUM", tag="ps_O")
                    nc.tensor.matmul(
                        out=ps_O[:], lhsT=w_sb[:], rhs=s_sb[:], start=True, stop=True
                    )

                    t0 = ep.tile([P, GN], f32, tag="t0")
                    nc.vector.tensor_tensor(
                        out=t0[:], in0=ps_O[:], in1=ngrp_t[:], op=mybir.AluOpType.mult
                    )
                    t1 = ep.tile([P, GN], f32, tag="t1")
                    nc.scalar.activation(
                        out=t1[:],
                        in_=t0[:],
                        func=mybir.ActivationFunctionType.Identity,
                        bias=bias_sb[:, :1],
                    )
                    t2 = ep.tile([P, GN], f32, tag="t2")
                    nc.vector.scalar_tensor_tensor(
                        out=t2[:],
                        in0=t1[:],
                        scalar=0.2,
                        in1=t1[:],
                        op0=mybir.AluOpType.mult,
                        op1=mybir.AluOpType.max,
                    )
                    nc.sync.dma_start(
                        out=outT_d[:, g * GN : (g + 1) * GN], in_=t2[:]
                    )
    nc.compile()
    return nc


def run_program(nc, in_maps):
    res = run_bass_kernel_spmd(nc, in_maps, list(range(NCORES)))
    outs = []
    for c in range(NCORES):
        outT = res.results[c]["outT"]  # [D, G*GN]
        outs.append(outT[:, :NODES_PER_CORE].T)
    return np.ascontiguousarray(np.concatenate(outs, axis=0))


def kernel(h, norm, weight, bias, src, dst):
    h = np.asarray(h, np.float32)
    norm = np.asarray(norm, np.float32)
    weight = np.asarray(weight, np.float32)
    bias = np.asarray(bias, np.float32)
    src = np.asarray(src, np.int32)
    dst = np.asarray(dst, np.int32)
    in_maps, meta = build_host_data(h, norm, weight, bias, src, dst)
    nc = build_program(meta)
    return run_program(nc, in_maps)

